# revision 15
# baseline (speedup 1.0000x reference)
"""DiT-X MoE block (top-2 of 4 experts + shared FFN) on 8 trn2 NeuronCores.

Strategy (globally load-balanced expert-major token schedule):
  * Routing is per-sample; the tiny gate network is evaluated on the host.
    The real work is tokenwise FFNs: for every token, the shared FFN plus
    the sample's two selected experts (minus the modality-masked token
    ranges: expert 1 skips wrist tokens [256,512), expert 2 skips head
    tokens [0,256) -- contiguous ranges we simply never schedule).
  * All (weight-set, token) work items across the WHOLE batch are laid out
    as one global list grouped by weight set (shared, e0..e3), chopped
    into uniform 512-token segments, and dealt S-per-core to the 8 cores.
    For the typical routing this is ~2048 token-units per core vs 2304
    for the naive one-sample-per-core split -- an ~11% compute cut -- and
    it is exactly balanced across cores.
  * Each core runs the same static program: for each of its S segments,
    stream that segment's (W1, W2) from DRAM, compute
    y = (gelu(x @ W1) [+ b1]) @ W2, scale by the per-token combine weight
    (gate weight; 1.0 for shared; 0.0 for padding), and DMA the partial
    out. The host scatter-adds partials into the (B, L, D) output --
    token work never needs cross-core reduction on device.
  * All matmuls in bf16 (fp32 PSUM accumulation); activations+weights are
    cast on host. Second-layer biases are folded in on the host.

Shapes (fixed): B=8, L=768, D=1024, H=4096, E=4, K=2.
"""

import math

import numpy as np
import ml_dtypes

B, L, D, H = 8, 768, 1024, 4096
NUM_EXPERTS, TOP_K = 4, 2
L3 = L // 3  # head / wrist / proprio segment length
KD = D // 128  # 8   k-tiles over D
KH = H // 128  # 32  k-tiles over H
TOK = 512      # tokens per segment (one full fp32 PSUM bank)
SHARED = NUM_EXPERTS  # weight-set id of the shared FFN

BF16 = ml_dtypes.bfloat16

_NC_CACHE = {}
_LAST = {"S": None, "with_b1": False, "mode": "uniform"}


def _gate_host(context_c, time_cond, gate_w, gate_b, time_w, time_b):
    """Replicates the reference gating math in fp32 numpy.

    Returns (topk_idx (B,2) int, topk_w (B,2) f32)."""
    full_agg = context_c.mean(axis=1)
    hp_agg = np.concatenate(
        [context_c[:, :L3], context_c[:, 2 * L3 :]], axis=1
    ).mean(axis=1)
    wp_agg = context_c[:, L3:].mean(axis=1)
    gate_in = np.concatenate([full_agg, hp_agg, wp_agg], axis=-1)

    logits = gate_in @ gate_w + gate_b
    silu = time_cond / (1.0 + np.exp(-time_cond))
    mod = silu @ time_w + time_b
    scale, shift = mod[:, :NUM_EXPERTS], mod[:, NUM_EXPERTS:]
    logits = logits * (1.0 + scale) + shift

    z = np.exp(logits - logits.max(axis=-1, keepdims=True))
    scores = z / z.sum(axis=-1, keepdims=True)

    # top-2, ties resolved to the lower index (jax.lax.top_k semantics)
    idx = np.argsort(-scores, axis=-1, kind="stable")[:, :TOP_K]
    w = np.take_along_axis(scores, idx, axis=-1)
    w = w / (w.sum(axis=-1, keepdims=True) + 1e-8)
    return idx, w.astype(np.float32)


def _expert_ranges(e):
    """Unmasked token ranges for expert e (modality masks are contiguous)."""
    if e == 1:
        return [(0, L3), (2 * L3, L)]   # skips wrist
    if e == 2:
        return [(L3, L)]                # skips head
    return [(0, L)]


def _build_segments(topk_idx, topk_w):
    """Global work list -> uniform 512-token segments grouped by weight set.

    Returns a list of segments; each is (set_id, [(b, t0, t1, wgt), ...])
    covering exactly TOK tokens (short tails padded via a (0,0,0,0.0) run).
    """
    # token runs per weight set
    runs = {sid: [] for sid in range(NUM_EXPERTS + 1)}
    for b in range(B):
        runs[SHARED].append((b, 0, L, 1.0))
        for k in range(TOP_K):
            e = int(topk_idx[b, k])
            for t0, t1 in _expert_ranges(e):
                runs[e].append((b, t0, t1, float(topk_w[b, k])))

    segs = []
    for sid in (SHARED, 3, 1, 2, 0):  # fixed order; groups same-set segs
        pend = [list(r) for r in runs[sid]]
        cur, room = [], TOK
        while pend:
            b, t0, t1, w = pend[0]
            take = min(room, t1 - t0)
            cur.append((b, t0, t0 + take, w))
            room -= take
            if t0 + take == t1:
                pend.pop(0)
            else:
                pend[0][1] = t0 + take
            if room == 0:
                segs.append((sid, cur))
                cur, room = [], TOK
        if cur:
            segs.append((sid, cur))  # padded below
    return segs


def _plan_m211(segs):
    """Try to pack segments as 8 cores x (single + same-set pair + single).

    Returns (singlesA, pairs, singlesC) -- each length 8; None entries are
    padding (set-agnostic) -- or None if infeasible."""
    if len(segs) > 4 * B:
        return None
    by_set = {}
    for seg in segs:
        by_set.setdefault(seg[0], []).append(seg)
    order = sorted(by_set, key=lambda sid: -len(by_set[sid]))
    pairs = []
    for sid in order:
        lst = by_set[sid]
        while len(lst) >= 2 and len(pairs) < B:
            pairs.append([lst.pop(), lst.pop()])
    while len(pairs) < B:
        sid = max(by_set, key=lambda k: len(by_set[k]), default=None)
        t = []
        if sid is not None:
            lst = by_set[sid]
            while lst and len(t) < 2:
                t.append(lst.pop())
        while len(t) < 2:
            t.append(None)
        pairs.append(t)
    leftovers = [s for sid in order for s in by_set[sid]]
    if len(leftovers) > 2 * B:
        return None
    singles = leftovers + [None] * (2 * B - len(leftovers))
    return singles[:B], pairs, singles[B:]


def _dedupe_ldweights(nc, mybir):
    """Drop an InstLdweights whose weights AP equals the immediately
    preceding PE weight load -- the stationary operand is still resident in
    the array, so the reload is pure overhead. Only sync-free duplicates are
    dropped; anything carrying waits/updates, or following a non-LDW/MM PE
    instruction, is kept."""
    PE = mybir.EngineType.PE
    dropped = 0
    for fn in nc.m.functions:
        for bb in fn.blocks:
            insts = bb.instructions
            keep = []
            prev_key = None
            for ins in insts:
                if ins.engine != PE:
                    keep.append(ins)
                    continue
                t = type(ins).__name__
                if t == "InstLdweights":
                    key = repr(ins.ins[0])
                    si = ins.sync_info
                    clean = not si or (not si.on_wait and not si.on_update)
                    if key == prev_key and clean:
                        dropped += 1
                        continue
                    prev_key = key
                    keep.append(ins)
                elif t == "InstMatmult":
                    keep.append(ins)
                else:
                    prev_key = None  # barrier/drain/branch: be conservative
                    keep.append(ins)
            if dropped and len(keep) != len(insts):
                bb.instructions = keep
    nc._dedupe_ldw_dropped = dropped
    return dropped


def _build_nc_phases(phases, n_slots, n_chunks, repeat=1, with_b1=None):
    """Phase program: each phase = (weight slot, list of contiguous chunk
    ids). Within a phase the slot's W1/W2 stream once and every stationary
    tile serves all the phase's moving chunks (PSUM chains interleaved)."""
    import concourse.mybir as mybir
    import concourse.tile as tile
    from concourse import bacc
    from contextlib import ExitStack

    if with_b1 is None:
        with_b1 = _LAST["with_b1"]
    C = n_chunks
    maxph = max(len(ch) for _, ch in phases)

    f32 = mybir.dt.float32
    bf16 = mybir.dt.bfloat16
    GELU = mybir.ActivationFunctionType.Gelu_apprx_tanh

    nc = bacc.Bacc(None, target_bir_lowering=False)
    xt_d = nc.declare_dram_parameter("xt", [128, KD, C, TOK], bf16, isOutput=False)
    w1_d = nc.declare_dram_parameter("w1", [n_slots, KH, 128, KD, 128], bf16, isOutput=False)
    w2_d = nc.declare_dram_parameter("w2", [n_slots, KD, 128, KH, 128], bf16, isOutput=False)
    wv_d = nc.declare_dram_parameter("wv", [128, C, TOK], f32, isOutput=False)
    b1_d = None
    if with_b1:
        b1_d = nc.declare_dram_parameter("b1", [128, n_slots, KH], f32, isOutput=False)
    y_d = nc.declare_dram_parameter("y", [128, KD, C, TOK], f32, isOutput=True)

    with tile.TileContext(nc) as tc, ExitStack() as ctx:
        const = ctx.enter_context(tc.tile_pool(name="const", bufs=1))
        xp = ctx.enter_context(tc.tile_pool(name="xp", bufs=2))
        w1p = ctx.enter_context(tc.tile_pool(name="w1p", bufs=3))
        w2p = ctx.enter_context(tc.tile_pool(name="w2p", bufs=2))
        hp = ctx.enter_context(tc.tile_pool(name="hp", bufs=2 if maxph == 1 else 1))
        op = ctx.enter_context(tc.tile_pool(name="op", bufs=3))
        psA = ctx.enter_context(tc.tile_pool(name="psA", bufs=2, space="PSUM"))
        psB = ctx.enter_context(tc.tile_pool(name="psB", bufs=2, space="PSUM"))

        wv = const.tile([128, C, TOK], f32)
        nc.sync.dma_start(wv, wv_d[:])
        b1 = None
        if with_b1:
            b1 = const.tile([128, n_slots, KH], f32)
            nc.sync.dma_start(b1, b1_d[:])

        for _rep in range(repeat):
            for slot, chs in phases:
                n = len(chs)
                c0 = chs[0]
                assert list(chs) == list(range(c0, c0 + n))
                xt = xp.tile([128, KD, n, TOK], bf16, tag="xt", name="xt")
                nc.sync.dma_start(xt, xt_d[:, :, c0 : c0 + n, :])
                hj = hp.tile([128, KH, n * TOK], bf16, tag="hj", name="hj")

                for m in range(KH):
                    w1t = w1p.tile([128, KD, 128], bf16, tag="w1t", name="w1t")
                    nc.sync.dma_start(w1t, w1_d[slot, m])
                    pss = [psA.tile([128, TOK], f32, tag=f"hps{ci}", name=f"hps{ci}")
                           for ci in range(n)]
                    for k in range(KD):
                        for ci in range(n):
                            nc.tensor.matmul(pss[ci], w1t[:, k, :], xt[:, k, ci, :],
                                             start=(k == 0), stop=(k == KD - 1))
                    for ci in range(n):
                        if with_b1:
                            nc.vector.tensor_scalar_add(
                                pss[ci], pss[ci], b1[:, slot, m : m + 1])
                        nc.scalar.activation(
                            hj[:, m, ci * TOK : (ci + 1) * TOK], pss[ci], GELU)

                for d in range(KD):
                    w2t = w2p.tile([128, KH, 128], bf16, tag="w2t", name="w2t")
                    nc.sync.dma_start(w2t, w2_d[slot, d])
                    pss = [psB.tile([128, TOK], f32, tag=f"yps{ci}", name=f"yps{ci}")
                           for ci in range(n)]
                    for k in range(KH):
                        for ci in range(n):
                            nc.tensor.matmul(
                                pss[ci], w2t[:, k, :],
                                hj[:, k, ci * TOK : (ci + 1) * TOK],
                                start=(k == 0), stop=(k == KH - 1))
                    for ci in range(n):
                        ot = op.tile([128, TOK], f32, tag=f"ot{ci}", name=f"ot{ci}")
                        nc.vector.tensor_mul(ot, pss[ci], wv[:, c0 + ci, :])
                        nc.sync.dma_start(y_d[:, d, c0 + ci, :], ot)

    nc.compile()
    _dedupe_ldweights(nc, mybir)
    return nc


def _build_nc(repeat=1, S=None, with_b1=None, mode=None):
    """Build the NEFF for the last-planned phase layout (or a uniform
    S-phase layout if S is given explicitly)."""
    if mode is None:
        mode = _LAST["mode"]
    if mode == "m211":
        phases = ((0, (0,)), (1, (1, 2)), (2, (3,)))
        return _build_nc_phases(phases, 3, 4, repeat=repeat, with_b1=with_b1)
    if S is None:
        S = _LAST["S"]
    phases = tuple((i, (i,)) for i in range(S))
    return _build_nc_phases(phases, S, S, repeat=repeat, with_b1=with_b1)


def _get_nc(S=None, with_b1=None, mode=None):
    if S is None:
        S = _LAST["S"]
    if with_b1 is None:
        with_b1 = _LAST["with_b1"]
    if mode is None:
        mode = _LAST["mode"]
    key = ("nc", mode, S, with_b1)
    if key not in _NC_CACHE:
        _NC_CACHE[key] = _build_nc(S=S, with_b1=with_b1, mode=mode)
    return _NC_CACHE[key]


def _seg_xt_wv(seg, context_c):
    """Per-segment device tensors (xt, wv) from a segment run list."""
    if seg is None:
        return np.zeros((128, KD, TOK), BF16), np.zeros((TOK,), np.float32)
    sid, spans = seg
    xs, wvs = [], []
    n = 0
    for b, t0, t1, w in spans:
        xs.append(context_c[b, t0:t1])
        wvs.append(np.full((t1 - t0,), w, np.float32))
        n += t1 - t0
    if n < TOK:
        xs.append(np.zeros((TOK - n, D), np.float32))
        wvs.append(np.zeros((TOK - n,), np.float32))
    xseg = np.concatenate(xs, axis=0)  # (TOK, D)
    xt = np.ascontiguousarray(
        xseg.T.reshape(KD, 128, TOK).transpose(1, 0, 2)
    ).astype(BF16)
    return xt, np.concatenate(wvs)


def kernel(
    context_c,
    time_cond,
    gate_w,
    gate_b,
    time_w,
    time_b,
    ew1,
    eb1,
    ew2,
    eb2,
    sw1,
    sb1,
    sw2,
    sb2,
):
    from concourse.bass_utils import run_bass_kernel_spmd

    context_c = np.asarray(context_c, dtype=np.float32)
    time_cond = np.asarray(time_cond, dtype=np.float32)

    topk_idx, topk_w = _gate_host(
        context_c, time_cond,
        np.asarray(gate_w, np.float32), np.asarray(gate_b, np.float32),
        np.asarray(time_w, np.float32), np.asarray(time_b, np.float32),
    )
    eb1 = np.asarray(eb1, np.float32)
    sb1 = np.asarray(sb1, np.float32)
    with_b1 = bool(np.any(eb1) or np.any(sb1))

    ew1 = np.asarray(ew1, np.float32)
    ew2 = np.asarray(ew2, np.float32)
    sw1 = np.asarray(sw1, np.float32)
    sw2 = np.asarray(sw2, np.float32)

    segs = _build_segments(topk_idx, topk_w)
    m211 = _plan_m211(segs)
    if m211 is not None:
        singlesA, pairs, singlesC = m211
        # per-core chunk lists [A, B0, B1, C] and slot map [setA, setB, setC]
        core_segs = [[singlesA[c]] + pairs[c] + [singlesC[c]] for c in range(B)]

        def _sid(x, default=SHARED):
            return x[0] if x is not None else default

        core_sets = []
        for c in range(B):
            sb = next((s[0] for s in pairs[c] if s is not None), SHARED)
            core_sets.append([_sid(singlesA[c]), sb, _sid(singlesC[c])])
        chunk2slot = [0, 1, 1, 2]
        S = 4
        _LAST["S"], _LAST["with_b1"], _LAST["mode"] = S, with_b1, "m211"
    else:
        S = max(1, math.ceil(len(segs) / B))
        padded = segs + [None] * (B * S - len(segs))
        core_segs = [padded[c * S : (c + 1) * S] for c in range(B)]
        core_sets = [
            [seg[0] if seg is not None else SHARED for seg in core_segs[c]]
            for c in range(B)
        ]
        chunk2slot = list(range(S))
        _LAST["S"], _LAST["with_b1"], _LAST["mode"] = S, with_b1, "uniform"

    # transpose/cast each distinct weight set once
    used = sorted({sid for sid, _ in segs})
    w1T, w2T, b1s = {}, {}, {}
    for sid in set(used) | {SHARED}:
        W1 = sw1 if sid == SHARED else ew1[sid]
        W2 = sw2 if sid == SHARED else ew2[sid]
        w1T[sid] = np.ascontiguousarray(
            W1.reshape(KD, 128, KH, 128).transpose(2, 1, 0, 3)
        ).astype(BF16)
        w2T[sid] = np.ascontiguousarray(
            W2.reshape(KH, 128, KD, 128).transpose(2, 1, 0, 3)
        ).astype(BF16)
        if with_b1:
            bvec = sb1 if sid == SHARED else eb1[sid]
            b1s[sid] = np.ascontiguousarray(bvec.reshape(KH, 128))

    in_maps = []
    for c in range(B):
        xts, wvs = [], []
        for seg in core_segs[c]:
            xt, wv = _seg_xt_wv(seg, context_c)
            xts.append(xt)
            wvs.append(wv)
        nchunks = len(core_segs[c])
        # [chunk, p, kd, t] -> [p, kd, chunk, t]
        xt_all = np.ascontiguousarray(np.stack(xts).transpose(1, 2, 0, 3))
        m = {
            "xt": xt_all,
            "w1": np.stack([w1T[sid] for sid in core_sets[c]]),
            "w2": np.stack([w2T[sid] for sid in core_sets[c]]),
            "wv": np.ascontiguousarray(
                np.broadcast_to(np.stack(wvs)[None], (128, nchunks, TOK))
            ),
        }
        if with_b1:
            # [p, slot, m] layout: per-H-channel scalar per weight slot
            m["b1"] = np.ascontiguousarray(
                np.stack([b1s[sid] for sid in core_sets[c]]).transpose(2, 0, 1)
            ).astype(np.float32)
        in_maps.append(m)

    nc = _get_nc(S=S, with_b1=with_b1)
    _NC_CACHE["last_in_maps"] = in_maps
    res = run_bass_kernel_spmd(nc, in_maps, core_ids=list(range(B)))

    # ---- host combine: scatter-add partials, fold second-layer biases
    out = np.zeros((B, L, D), np.float32)
    for c in range(B):
        y = res.results[c]["y"]  # [p, d, chunk, t]
        for i, seg in enumerate(core_segs[c]):
            if seg is None:
                continue
            part = np.ascontiguousarray(
                y[:, :, i, :].transpose(2, 1, 0)).reshape(TOK, D)
            off = 0
            for b, t0, t1, w in seg[1]:
                out[b, t0:t1] += part[off : off + (t1 - t0)]
                off += t1 - t0

    eb2 = np.asarray(eb2, np.float32)
    sb2 = np.asarray(sb2, np.float32)
    mask = np.ones((NUM_EXPERTS, L), dtype=np.float32)
    mask[1, L3 : 2 * L3] = 0.0
    mask[2, :L3] = 0.0
    for b in range(B):
        bias = np.broadcast_to(sb2[None, :], (L, D)).copy()
        for k in range(TOP_K):
            e = int(topk_idx[b, k])
            bias += (topk_w[b, k] * mask[e])[:, None] * eb2[e][None, :]
        out[b] += bias
    return out
